# revision 28
# baseline (speedup 1.0000x reference)
"""GQA (= full MHA) attention kernel for 8 Trainium2 NeuronCores.

Problem: B=2, T=2048 queries, K=2048 keys, H=16 heads, D=128, fp32.
The reference's "group" reshape is a no-op view: this is plain softmax
attention per (batch, head). 32 independent (b,h) problems -> 4 per core.

Per-core device program (SPMD, different input slices per core):
  - Host pre-transposes Q,K to (d, t)/(d, k) layout, V to k-blocked
    (kk, j*D+d) layout, casts all to fp16.
  - Per (pair, t-slice of 512): for each of 8 j-pairs (j = 128-key block):
      S^T halves = K_j^T.T @ Q^T into a (128, 1024) 2-bank PSUM tile,
      ONE 1024-elem exp on the scalar engine -> P pair tile (fp16),
      two PV matmuls accumulate O^T (d, t) in PSUM over all 16 j,
      DVE add-tree accumulates P over j -> acc (128, 512),
    then one ones-matmul on acc gives the softmax denominator l (1, 512).
  - O^T drains via DVE as fp16; host does transpose + divide by l.

Engine balance per core (cost model): ACT ~133us (bottleneck, 128x
1024-elem exp), PE ~113us (512+16+1 matmuls of 512 cols), DVE ~90us.
"""

import numpy as np
import ml_dtypes

import concourse.bacc as bacc
import concourse.tile as tile
import concourse.mybir as mybir
from concourse.bass_utils import run_bass_kernel_spmd

B = 2
T = 2048
KSEQ = 2048
H = 16
D = 128
N_CORES = 8
PAIRS = (B * H) // N_CORES  # 4 (b,h) pairs per core
TSLICE = 512
NS = T // TSLICE  # 4
KTILES = KSEQ // 128  # 16
# j-groups per slice: one exp instruction per group (bigger groups
# amortize the ~185ns ACT per-instruction overhead; 3 banks is the max
# S-PSUM tile with double buffering + O/l banks: 2*3 + 1 + 1 = 8)
GROUPS = (2, 3, 3, 3, 3, 2)
GOFF = tuple(sum(GROUPS[:i]) for i in range(len(GROUPS)))  # j offsets
GMAX = max(GROUPS)
SCALE = 1.0 / float(np.sqrt(D))

f32 = mybir.dt.float32
f16 = mybir.dt.float16

_cache = {}


def _build(repeat=1, dyn_loop=1):
    key = ("nc", repeat, dyn_loop)
    if key in _cache:
        return _cache[key]
    nc = bacc.Bacc(None, target_bir_lowering=False)
    with tile.TileContext(nc) as tc:
        with tc.tile_pool(name="dram", bufs=1, space="DRAM") as dram:
            qt_in = dram.tile([PAIRS, 128, T], f16, kind="ExternalInput",
                              name="qt_in", uniquify=False)
            kt_in = dram.tile([PAIRS, 128, KSEQ], f16, kind="ExternalInput",
                              name="kt_in", uniquify=False)
            v_in = dram.tile([PAIRS, 128, KTILES * D], f16,
                             kind="ExternalInput", name="v_in",
                             uniquify=False)
            ot_out = dram.tile([PAIRS, 128, T], f16, kind="ExternalOutput",
                               name="ot_out", uniquify=False)
            # per-slice partial denominator: k-partials summed over j only;
            # the final 128-way partition sum happens on the host
            l_out = dram.tile([PAIRS, NS, 128, TSLICE], f16,
                              kind="ExternalOutput", name="l_out",
                              uniquify=False)
            if dyn_loop > 1:
                with tc.For_i(0, dyn_loop, 1):
                    _attn_body(nc, tc, qt_in, kt_in, v_in, ot_out, l_out,
                               repeat)
            else:
                _attn_body(nc, tc, qt_in, kt_in, v_in, ot_out, l_out, repeat)
    nc.compile()
    _cache[key] = nc
    return nc


def _attn_body(nc, tc, qt_in, kt_in, v_in, ot_out, l_out, repeat):
    with (
        tc.tile_pool(name="qkv", bufs=PAIRS) as qkv,
        tc.tile_pool(name="ptp", bufs=6) as ptp,
        tc.tile_pool(name="red", bufs=4) as red,
        tc.tile_pool(name="drain", bufs=4) as drp,
        tc.tile_pool(name="ps_s", bufs=2, space="PSUM") as ps_s,
        tc.tile_pool(name="ps_o", bufs=2, space="PSUM") as ps_o,
    ):
        def load_pair(p, chunked=False):
            qt = qkv.tile([128, T], f16, tag="qt", name=f"qt_{p}")
            kt = qkv.tile([128, KSEQ], f16, tag="kt", name=f"kt_{p}")
            v = qkv.tile([128, KTILES * D], f16, tag="v", name=f"v_{p}")
            if chunked:
                # the first exp group needs only K j-blocks 0-2 and the
                # first Q slice: ship those tiny chunks first so compute
                # starts as early as possible
                c = GROUPS[0] * 128
                nc.sync.dma_start(out=kt[:, :c], in_=kt_in[p, :, :c])
                nc.sync.dma_start(out=qt[:, :TSLICE], in_=qt_in[p, :, :TSLICE])
                nc.sync.dma_start(out=kt[:, c:], in_=kt_in[p, :, c:])
                nc.sync.dma_start(out=v[:], in_=v_in[p])
                nc.sync.dma_start(out=qt[:, TSLICE:], in_=qt_in[p, :, TSLICE:])
            else:
                nc.sync.dma_start(out=qt[:], in_=qt_in[p])
                nc.sync.dma_start(out=kt[:], in_=kt_in[p])
                nc.sync.dma_start(out=v[:], in_=v_in[p])
            return qt, kt, v

        # flat step list: one step = one j-group of one (pair, slice);
        # software-pipelined by one step so the PE never sits behind a
        # wait-on-ACT in its FIFO: step i issues S-matmuls + exp for i,
        # then PV matmuls + denominator adds for step i-1.
        slices = [(p, s) for _ in range(repeat)
                  for p in range(PAIRS) for s in range(NS)]
        steps = [(si, p, s, gi) for si, (p, s) in enumerate(slices)
                 for gi in range(len(GROUPS))]
        pair_tiles = {}
        for p in range(PAIRS):
            pair_tiles[p] = load_pair(p, chunked=(p == 0))
        state = {}  # si -> dict with po tile, running acc tile
        pend = []   # completed (S, exp) steps whose consumers are pending
        n_steps = len(steps)
        last_si = len(slices) - 1
        LAG = 2  # consumer block trails by 2 steps: its sems are long
        # propagated by the time the PE FIFO reaches it (no head-blocking)
        for i in range(n_steps + LAG):
            prev = None
            if i >= LAG:
                prev = pend.pop(0)
            if prev is not None:
                si_, p_, s_, gi_, pt_, v_ = prev
                ts_ = slice(s_ * TSLICE, (s_ + 1) * TSLICE)
                glen_, joff_ = GROUPS[gi_], GOFF[gi_]
                st = state.setdefault(si_, {})
                if gi_ == 0:
                    st["po"] = ps_o.tile([128, TSLICE], f32, tag="o",
                                         name=f"po_{si_}")
                po = st["po"]
                for jx in range(glen_):
                    j = joff_ + jx
                    nc.tensor.matmul(
                        po[:], v_[:, j * D:(j + 1) * D],
                        pt_[:, jx * TSLICE:(jx + 1) * TSLICE],
                        start=(j == 0), stop=(j == KTILES - 1))
                # denominator: running per-partition sum on DVE
                # (alternating output buffers, no in-place hazard)
                for jx in range(glen_):
                    j = joff_ + jx
                    pslice = pt_[:, jx * TSLICE:(jx + 1) * TSLICE]
                    if j == 0:
                        st["acc"] = None
                        st["pend"] = pslice
                        continue
                    nacc = red.tile([128, TSLICE], f16, tag=f"r{j % 3}",
                                    name=f"acc_{si_}_{j}")
                    if st["acc"] is None:
                        nc.vector.tensor_add(nacc[:], st["pend"], pslice)
                    else:
                        nc.vector.tensor_add(nacc[:], st["acc"][:], pslice)
                    st["acc"] = nacc
                if gi_ == len(GROUPS) - 1:
                    acc = st["acc"]
                    nc.sync.dma_start(out=l_out[p_, s_], in_=acc[:])
                    osb = drp.tile([128, TSLICE], f16, tag="osb",
                                   name=f"osb_{si_}")
                    half = TSLICE // 2
                    hs0 = slice(s_ * TSLICE, s_ * TSLICE + half)
                    hs1 = slice(s_ * TSLICE + half, (s_ + 1) * TSLICE)
                    nc.vector.tensor_copy(osb[:, :half], po[:, :half])
                    nc.sync.dma_start(out=ot_out[p_, :, hs0],
                                      in_=osb[:, :half])
                    if si_ == last_si:
                        # tail: ACT is idle by now, keep DVE off the path
                        nc.scalar.copy(osb[:, half:], po[:, half:])
                    else:
                        nc.vector.tensor_copy(osb[:, half:], po[:, half:])
                    nc.sync.dma_start(out=ot_out[p_, :, hs1],
                                      in_=osb[:, half:])
                    del state[si_]
            if i < n_steps:
                si, p, s, gi = steps[i]
                qt, kt, v = pair_tiles[p]
                ts = slice(s * TSLICE, (s + 1) * TSLICE)
                glen, joff = GROUPS[gi], GOFF[gi]
                gsz = glen * TSLICE
                ps = ps_s.tile([128, GMAX * TSLICE], f32, tag="s",
                               name=f"ps_{si}_{gi}")
                # high priority: the S->exp chain is the kernel's critical
                # path; the scheduler must prefer S-matmuls over the PV
                # matmuls whenever both are ready on the PE
                with tc.high_priority(offset=1 << 20):
                    for jx in range(glen):
                        j = joff + jx
                        nc.tensor.matmul(
                            ps[:, jx * TSLICE:(jx + 1) * TSLICE],
                            kt[:, j * 128:(j + 1) * 128],
                            qt[:, ts], start=True, stop=True)
                    pt = ptp.tile([128, GMAX * TSLICE], f16, tag="pt",
                                  name=f"pt_{si}_{gi}")
                    nc.scalar.activation(
                        pt[:, :gsz], ps[:, :gsz],
                        mybir.ActivationFunctionType.Exp, scale=SCALE)
                pend.append((si, p, s, gi, pt, v))


def _prep(query, key, value):
    """Host-side shard + layout + cast. Returns per-core input maps."""
    q4 = query.reshape(B, T, H, D)
    # (b,h,d,t) so each pair's Q^T is (128, T) with d on partitions
    qT = np.ascontiguousarray(q4.transpose(0, 2, 3, 1)).reshape(B * H, D, T)
    kT = np.ascontiguousarray(key.transpose(0, 2, 3, 1)).reshape(B * H, D, KSEQ)
    # V: (bh, kk, j*D+d) with kk = k % 128, j = k // 128
    v = value.transpose(0, 2, 1, 3).reshape(B * H, KTILES, 128, D)
    v = np.ascontiguousarray(v.transpose(0, 2, 1, 3)).reshape(
        B * H, 128, KTILES * D)
    qT = qT.astype(np.float16)
    kT = kT.astype(np.float16)
    v = v.astype(np.float16)
    in_maps = []
    for c in range(N_CORES):
        sl = slice(c * PAIRS, (c + 1) * PAIRS)
        in_maps.append({
            "qt_in": np.ascontiguousarray(qT[sl]),
            "kt_in": np.ascontiguousarray(kT[sl]),
            "v_in": np.ascontiguousarray(v[sl]),
        })
    return in_maps


def _post(results):
    """Gather per-core outputs, normalize, restore (B, T, H*D) fp32."""
    ot = np.stack([r["ot_out"] for r in results])  # (8, PAIRS, D, T) f16
    # (8, PAIRS, NS, 128, TSLICE) f16 k-partials -> sum the 128 partitions
    l = np.stack([r["l_out"] for r in results])
    ot = ot.reshape(B * H, D, T).astype(np.float32)
    l = l.astype(np.float32).sum(axis=3).reshape(B * H, T)
    o = ot.transpose(0, 2, 1) / l[:, :, None]      # (BH, T, D)
    o = o.reshape(B, H, T, D).transpose(0, 2, 1, 3).reshape(B, T, H * D)
    return np.ascontiguousarray(o.astype(np.float32))


def kernel(query, key, value):
    nc = _build()
    in_maps = _prep(query, key, value)
    res = run_bass_kernel_spmd(nc, in_maps, core_ids=list(range(N_CORES)))
    return _post(res.results)


if __name__ == "__main__":
    rng = np.random.default_rng(0)
    q = rng.standard_normal((B, T, H * D), dtype=np.float32)
    k = rng.standard_normal((B, KSEQ, H, D), dtype=np.float32)
    v = rng.standard_normal((B, KSEQ, H, D), dtype=np.float32)
    out = kernel(q, k, v)
    print("out", out.shape, out.dtype)


# revision 31
# speedup vs baseline: 1.0955x; 1.0955x over previous
"""GQA (= full MHA) attention kernel for 8 Trainium2 NeuronCores.

Problem: B=2, T=2048 queries, K=2048 keys, H=16 heads, D=128, fp32.
The reference's "group" reshape is a no-op view: this is plain softmax
attention per (batch, head). 32 independent (b,h) problems -> 4 per core.

Per-core device program (SPMD, different input slices per core):
  - Host pre-transposes Q,K to (d, t)/(d, k) layout, V to k-blocked
    (kk, j*D+d) layout, casts all to fp16.
  - Per (pair, t-slice of 512): for each of 8 j-pairs (j = 128-key block):
      S^T halves = K_j^T.T @ Q^T into a (128, 1024) 2-bank PSUM tile,
      ONE 1024-elem exp on the scalar engine -> P pair tile (fp16),
      two PV matmuls accumulate O^T (d, t) in PSUM over all 16 j,
      DVE add-tree accumulates P over j -> acc (128, 512),
    then one ones-matmul on acc gives the softmax denominator l (1, 512).
  - O^T drains via DVE as fp16; host does transpose + divide by l.

Engine balance per core (cost model): ACT ~133us (bottleneck, 128x
1024-elem exp), PE ~113us (512+16+1 matmuls of 512 cols), DVE ~90us.
"""

import numpy as np
import ml_dtypes

import concourse.bacc as bacc
import concourse.tile as tile
import concourse.mybir as mybir
from concourse.bass_utils import run_bass_kernel_spmd

B = 2
T = 2048
KSEQ = 2048
H = 16
D = 128
N_CORES = 8
PAIRS = (B * H) // N_CORES  # 4 (b,h) pairs per core
TSLICE = 512
NS = T // TSLICE  # 4
KTILES = KSEQ // 128  # 16
# j-groups per slice: one exp instruction per group (bigger groups
# amortize the ~185ns ACT per-instruction overhead; 3 banks is the max
# S-PSUM tile with double buffering + O/l banks: 2*3 + 1 + 1 = 8)
GROUPS = (2, 3, 3, 3, 3, 2)
GOFF = tuple(sum(GROUPS[:i]) for i in range(len(GROUPS)))  # j offsets
GMAX = max(GROUPS)
SCALE = 1.0 / float(np.sqrt(D))

f32 = mybir.dt.float32
f16 = mybir.dt.float16

_cache = {}


def _build(repeat=1, dyn_loop=1):
    key = ("nc", repeat, dyn_loop)
    if key in _cache:
        return _cache[key]
    nc = bacc.Bacc(None, target_bir_lowering=False)
    with tile.TileContext(nc) as tc:
        with tc.tile_pool(name="dram", bufs=1, space="DRAM") as dram:
            qt_in = dram.tile([PAIRS, 128, T], f16, kind="ExternalInput",
                              name="qt_in", uniquify=False)
            kt_in = dram.tile([PAIRS, 128, KSEQ], f16, kind="ExternalInput",
                              name="kt_in", uniquify=False)
            v_in = dram.tile([PAIRS, 128, KTILES * D], f16,
                             kind="ExternalInput", name="v_in",
                             uniquify=False)
            ot_out = dram.tile([PAIRS, 128, T], f16, kind="ExternalOutput",
                               name="ot_out", uniquify=False)
            # per-slice partial denominator: k-partials summed over j only;
            # the final 128-way partition sum happens on the host
            l_out = dram.tile([PAIRS, NS, 128, TSLICE], f16,
                              kind="ExternalOutput", name="l_out",
                              uniquify=False)
            if dyn_loop > 1:
                with tc.For_i(0, dyn_loop, 1):
                    _attn_body(nc, tc, qt_in, kt_in, v_in, ot_out, l_out,
                               repeat)
            else:
                _attn_body(nc, tc, qt_in, kt_in, v_in, ot_out, l_out, repeat)
    nc.compile()
    _cache[key] = nc
    return nc


def _attn_body(nc, tc, qt_in, kt_in, v_in, ot_out, l_out, repeat):
    with (
        tc.tile_pool(name="qkv", bufs=PAIRS) as qkv,
        tc.tile_pool(name="ptp", bufs=6) as ptp,
        tc.tile_pool(name="red", bufs=4) as red,
        tc.tile_pool(name="drain", bufs=4) as drp,
        tc.tile_pool(name="ps_s", bufs=2, space="PSUM") as ps_s,
        tc.tile_pool(name="ps_o", bufs=2, space="PSUM") as ps_o,
    ):
        def load_pair(p, chunked=False):
            qt = qkv.tile([128, T], f16, tag="qt", name=f"qt_{p}")
            kt = qkv.tile([128, KSEQ], f16, tag="kt", name=f"kt_{p}")
            v = qkv.tile([128, KTILES * D], f16, tag="v", name=f"v_{p}")
            if chunked:
                # the first exp group needs only K j-blocks 0-2 and the
                # first Q slice: ship those tiny chunks first so compute
                # starts as early as possible
                c = GROUPS[0] * 128
                nc.sync.dma_start(out=kt[:, :c], in_=kt_in[p, :, :c])
                nc.sync.dma_start(out=qt[:, :TSLICE], in_=qt_in[p, :, :TSLICE])
                nc.sync.dma_start(out=kt[:, c:], in_=kt_in[p, :, c:])
                nc.sync.dma_start(out=v[:], in_=v_in[p])
                nc.sync.dma_start(out=qt[:, TSLICE:], in_=qt_in[p, :, TSLICE:])
            else:
                nc.sync.dma_start(out=qt[:], in_=qt_in[p])
                nc.sync.dma_start(out=kt[:], in_=kt_in[p])
                nc.sync.dma_start(out=v[:], in_=v_in[p])
            return qt, kt, v

        # flat step list: one step = one j-group of one (pair, slice);
        # software-pipelined by one step so the PE never sits behind a
        # wait-on-ACT in its FIFO: step i issues S-matmuls + exp for i,
        # then PV matmuls + denominator adds for step i-1.
        slices = [(p, s) for _ in range(repeat)
                  for p in range(PAIRS) for s in range(NS)]
        steps = [(si, p, s, gi) for si, (p, s) in enumerate(slices)
                 for gi in range(len(GROUPS))]
        pair_tiles = {}
        for p in range(PAIRS):
            pair_tiles[p] = load_pair(p, chunked=(p == 0))
        state = {}  # si -> dict with po tile, running acc tile
        pend = []   # completed (S, exp) steps whose consumers are pending
        n_steps = len(steps)
        last_si = len(slices) - 1
        LAG = 2  # consumer block trails by 2 steps: its sems are long
        # propagated by the time the PE FIFO reaches it (no head-blocking)
        for i in range(n_steps + LAG):
            prev = None
            if i >= LAG:
                prev = pend.pop(0)
            if prev is not None:
                si_, p_, s_, gi_, pt_, v_ = prev
                ts_ = slice(s_ * TSLICE, (s_ + 1) * TSLICE)
                glen_, joff_ = GROUPS[gi_], GOFF[gi_]
                st = state.setdefault(si_, {})
                if gi_ == 0:
                    st["po"] = ps_o.tile([128, TSLICE], f32, tag="o",
                                         name=f"po_{si_}")
                po = st["po"]
                for jx in range(glen_):
                    j = joff_ + jx
                    nc.tensor.matmul(
                        po[:], v_[:, j * D:(j + 1) * D],
                        pt_[:, jx * TSLICE:(jx + 1) * TSLICE],
                        start=(j == 0), stop=(j == KTILES - 1))
                # denominator: running per-partition sum on DVE
                # (alternating output buffers, no in-place hazard)
                for jx in range(glen_):
                    j = joff_ + jx
                    pslice = pt_[:, jx * TSLICE:(jx + 1) * TSLICE]
                    if j == 0:
                        st["acc"] = None
                        st["pend"] = pslice
                        continue
                    # final acc gets its own tag: its buffer is held by the
                    # outgoing DMA and must never gate the next slice's chain
                    tag = "accout" if j == KTILES - 1 else f"r{j % 3}"
                    nacc = red.tile([128, TSLICE], f16, tag=tag,
                                    name=f"acc_{si_}_{j}")
                    if st["acc"] is None:
                        nc.vector.tensor_add(nacc[:], st["pend"], pslice)
                    else:
                        nc.vector.tensor_add(nacc[:], st["acc"][:], pslice)
                    st["acc"] = nacc
                if gi_ == len(GROUPS) - 1:
                    acc = st["acc"]
                    nc.sync.dma_start(out=l_out[p_, s_], in_=acc[:])
                    osb = drp.tile([128, TSLICE], f16, tag="osb",
                                   name=f"osb_{si_}")
                    half = TSLICE // 2
                    hs0 = slice(s_ * TSLICE, s_ * TSLICE + half)
                    hs1 = slice(s_ * TSLICE + half, (s_ + 1) * TSLICE)
                    if si_ == last_si:
                        # tail: ACT is idle by now, keep DVE (still busy
                        # with the denominator chain) off the drain path
                        nc.scalar.copy(osb[:, :half], po[:, :half])
                    else:
                        nc.vector.tensor_copy(osb[:, :half], po[:, :half])
                    nc.sync.dma_start(out=ot_out[p_, :, hs0],
                                      in_=osb[:, :half])
                    if si_ == last_si:
                        nc.scalar.copy(osb[:, half:], po[:, half:])
                    else:
                        nc.vector.tensor_copy(osb[:, half:], po[:, half:])
                    nc.sync.dma_start(out=ot_out[p_, :, hs1],
                                      in_=osb[:, half:])
                    del state[si_]
            if i < n_steps:
                si, p, s, gi = steps[i]
                qt, kt, v = pair_tiles[p]
                ts = slice(s * TSLICE, (s + 1) * TSLICE)
                glen, joff = GROUPS[gi], GOFF[gi]
                gsz = glen * TSLICE
                ps = ps_s.tile([128, GMAX * TSLICE], f32, tag="s",
                               name=f"ps_{si}_{gi}")
                for jx in range(glen):
                    j = joff + jx
                    nc.tensor.matmul(
                        ps[:, jx * TSLICE:(jx + 1) * TSLICE],
                        kt[:, j * 128:(j + 1) * 128],
                        qt[:, ts], start=True, stop=True)
                pt = ptp.tile([128, GMAX * TSLICE], f16, tag="pt",
                              name=f"pt_{si}_{gi}")
                nc.scalar.activation(
                    pt[:, :gsz], ps[:, :gsz],
                    mybir.ActivationFunctionType.Exp, scale=SCALE)
                pend.append((si, p, s, gi, pt, v))


def _prep(query, key, value):
    """Host-side shard + layout + cast. Returns per-core input maps."""
    q4 = query.reshape(B, T, H, D)
    # (b,h,d,t) so each pair's Q^T is (128, T) with d on partitions
    qT = np.ascontiguousarray(q4.transpose(0, 2, 3, 1)).reshape(B * H, D, T)
    kT = np.ascontiguousarray(key.transpose(0, 2, 3, 1)).reshape(B * H, D, KSEQ)
    # V: (bh, kk, j*D+d) with kk = k % 128, j = k // 128
    v = value.transpose(0, 2, 1, 3).reshape(B * H, KTILES, 128, D)
    v = np.ascontiguousarray(v.transpose(0, 2, 1, 3)).reshape(
        B * H, 128, KTILES * D)
    qT = qT.astype(np.float16)
    kT = kT.astype(np.float16)
    v = v.astype(np.float16)
    in_maps = []
    for c in range(N_CORES):
        sl = slice(c * PAIRS, (c + 1) * PAIRS)
        in_maps.append({
            "qt_in": np.ascontiguousarray(qT[sl]),
            "kt_in": np.ascontiguousarray(kT[sl]),
            "v_in": np.ascontiguousarray(v[sl]),
        })
    return in_maps


def _post(results):
    """Gather per-core outputs, normalize, restore (B, T, H*D) fp32."""
    ot = np.stack([r["ot_out"] for r in results])  # (8, PAIRS, D, T) f16
    # (8, PAIRS, NS, 128, TSLICE) f16 k-partials -> sum the 128 partitions
    l = np.stack([r["l_out"] for r in results])
    ot = ot.reshape(B * H, D, T).astype(np.float32)
    l = l.astype(np.float32).sum(axis=3).reshape(B * H, T)
    o = ot.transpose(0, 2, 1) / l[:, :, None]      # (BH, T, D)
    o = o.reshape(B, H, T, D).transpose(0, 2, 1, 3).reshape(B, T, H * D)
    return np.ascontiguousarray(o.astype(np.float32))


def kernel(query, key, value):
    nc = _build()
    in_maps = _prep(query, key, value)
    res = run_bass_kernel_spmd(nc, in_maps, core_ids=list(range(N_CORES)))
    return _post(res.results)


if __name__ == "__main__":
    rng = np.random.default_rng(0)
    q = rng.standard_normal((B, T, H * D), dtype=np.float32)
    k = rng.standard_normal((B, KSEQ, H, D), dtype=np.float32)
    v = rng.standard_normal((B, KSEQ, H, D), dtype=np.float32)
    out = kernel(q, k, v)
    print("out", out.shape, out.dtype)
